# revision 13
# baseline (speedup 1.0000x reference)
"""Trainium2 Bass kernel for the attention-LSTM decoder step.

Model (B=64, S=128, H=1024, E=512, V=32000):
    emb lookup -> additive attention over encoder_outputs -> LSTMCell
    -> vocab projection -> log_softmax
Returns (output [B,V], hidden [B,H], cell [B,H], attn_weights [B,S]).

Sharding over 8 NeuronCores:
  - attention: data-parallel over batch (8 rows/core, encoder shard 4.2MB)
  - LSTM: sharded over hidden dim (each core computes a 128-wide slice of
    all four gates; w_ih/w_hh column shards)
  - vocab projection: sharded over V (4000 vocab rows/core)
  - collectives: AllGather x (LSTM input features), AllGather hiddenT,
    AllGather per-core log-softmax stats (max, sumexp).
Weights are pre-transposed on the host so every DMA load is contiguous.
"""

import numpy as np
from contextlib import ExitStack

import concourse.bass as bass
import concourse.bacc as bacc
import concourse.mybir as mybir
from concourse.tile import TileContext
from concourse.masks import make_identity
from concourse.bass_utils import run_bass_kernel_spmd

F32 = mybir.dt.float32
F32R = mybir.dt.float32r
I32 = mybir.dt.int32

V, H, E, B, S = 32000, 1024, 512, 64, 128
NCORE = 8
BL = B // NCORE          # 8 batch rows per core (attention)
HL = H // NCORE          # 128-wide hidden slice per core (LSTM)
VL = V // NCORE          # 4000 vocab rows per core (projection)
EH = E + H               # 1536 LSTM input features
KC_X = EH // 128         # 12 k-chunks for w_ih contraction
KC_H = H // 128          # 8 k-chunks for w_hh contraction
NT = 8                   # projection free-dim tiles
NTW = VL // NT           # 500 columns per projection tile

_prog_cache = {}


def _r(ap):
    """View an fp32 AP as float32r for full-rate PE matmuls."""
    return ap.bitcast(F32R)


def build_program():
    nc = bacc.Bacc(None, num_devices=NCORE)

    # ---- per-core external inputs ----
    tok = nc.declare_dram_parameter("tok", [BL, 1], I32, isOutput=False)
    emb_h = nc.declare_dram_parameter("emb", [V, E], F32, isOutput=False)
    enc_h = nc.declare_dram_parameter("enc", [BL, S, H], F32, isOutput=False)
    ph_row = nc.declare_dram_parameter("ph_row", [BL, H], F32, isOutput=False)
    phT_h = nc.declare_dram_parameter("phT", [H, B], F32, isOutput=False)
    pcT_h = nc.declare_dram_parameter("pcT_loc", [HL, B], F32, isOutput=False)
    we_h = nc.declare_dram_parameter("we_b", [S, H], F32, isOutput=False)
    wh_h = nc.declare_dram_parameter("wh_b", [BL, H], F32, isOutput=False)
    ab_h = nc.declare_dram_parameter("ab_b", [BL, 1], F32, isOutput=False)
    wih_h = nc.declare_dram_parameter("wihT_loc", [EH, 4 * HL], F32, isOutput=False)
    whh_h = nc.declare_dram_parameter("whhT_loc", [H, 4 * HL], F32, isOutput=False)
    bias_h = nc.declare_dram_parameter("bias_loc", [HL, 4], F32, isOutput=False)
    owt_h = nc.declare_dram_parameter("owTb_loc", [H + 1, VL], F32, isOutput=False)
    ones_h = nc.declare_dram_parameter("ones", [1, B], F32, isOutput=False)

    # ---- per-core external outputs ----
    attn_o = nc.declare_dram_parameter("attn_out", [BL, S], F32, isOutput=True)
    hid_o = nc.declare_dram_parameter("hidden_out", [HL, B], F32, isOutput=True)
    cell_o = nc.declare_dram_parameter("cell_out", [HL, B], F32, isOutput=True)
    log_o = nc.declare_dram_parameter("logits_out", [B, VL], F32, isOutput=True)

    # ---- internal DRAM for collectives ----
    xcon = nc.dram_tensor("xcon", [EH, BL], F32)
    XG = nc.dram_tensor("XG", [NCORE, EH, BL], F32, addr_space="Shared")
    hcon = nc.dram_tensor("hcon", [HL, B], F32)
    HG = nc.dram_tensor("HG", [NCORE, HL, B], F32, addr_space="Shared")
    scon = nc.dram_tensor("scon", [B, 2], F32)
    SG = nc.dram_tensor("SG", [NCORE, B, 2], F32, addr_space="Shared")

    groups = [list(range(NCORE))]

    with TileContext(nc) as tc, ExitStack() as ctx:
        cpool = ctx.enter_context(tc.tile_pool(name="const", bufs=1))
        wpool = ctx.enter_context(tc.tile_pool(name="wts", bufs=1))
        spool = ctx.enter_context(tc.tile_pool(name="scr", bufs=1))

        # ---------- constants ----------
        ident = cpool.tile([128, 128], F32, tag="ident")
        make_identity(nc, ident[:])
        ones_row = cpool.tile([1, B], F32R, tag="ones")
        nc.sync.dma_start(out=ones_row[:], in_=ones_h[:].bitcast(F32R))

        # ---------- persistent loads ----------
        enc_t = wpool.tile([S, BL * H], F32, tag="enc")          # 32KB/p
        for b in range(BL):
            nc.sync.dma_start(out=enc_t[:, b * H:(b + 1) * H], in_=enc_h[b])
        we_t = wpool.tile([S, H], F32, tag="we")
        nc.sync.dma_start(out=we_t[:], in_=we_h[:])
        ph_t = wpool.tile([BL, H], F32, tag="ph")
        nc.sync.dma_start(out=ph_t[:], in_=ph_row[:])
        wh_t = wpool.tile([BL, H], F32, tag="wh")
        nc.sync.dma_start(out=wh_t[:], in_=wh_h[:])
        ab_t = wpool.tile([BL, 1], F32, tag="ab")
        nc.sync.dma_start(out=ab_t[:], in_=ab_h[:])
        idx_t = wpool.tile([BL, 1], I32, tag="idx")
        nc.sync.dma_start(out=idx_t[:], in_=tok[:])
        phT_t = wpool.tile([128, KC_H * B], F32R, tag="phT")
        for c in range(KC_H):
            nc.sync.dma_start(out=phT_t[:, c * B:(c + 1) * B],
                              in_=phT_h[c * 128:(c + 1) * 128, :].bitcast(F32R))
        pcT_t = wpool.tile([HL, B], F32, tag="pcT")
        nc.sync.dma_start(out=pcT_t[:], in_=pcT_h[:])
        wih_t = wpool.tile([128, KC_X * 4 * HL], F32R, tag="wih")  # 24KB/p
        for c in range(KC_X):
            nc.sync.dma_start(out=wih_t[:, c * 4 * HL:(c + 1) * 4 * HL],
                              in_=wih_h[c * 128:(c + 1) * 128, :].bitcast(F32R))
        whh_t = wpool.tile([128, KC_H * 4 * HL], F32R, tag="whh")  # 16KB/p
        for c in range(KC_H):
            nc.sync.dma_start(out=whh_t[:, c * 4 * HL:(c + 1) * 4 * HL],
                              in_=whh_h[c * 128:(c + 1) * 128, :].bitcast(F32R))
        bias_t = wpool.tile([HL, 4], F32, tag="bias")
        nc.sync.dma_start(out=bias_t[:], in_=bias_h[:])
        wb_t = wpool.tile([1, VL], F32R, tag="wbias")
        nc.sync.dma_start(out=wb_t[:], in_=owt_h[H:H + 1, :].bitcast(F32R))

        # embedding gather (indirect DMA, row indices)
        emb_t = wpool.tile([BL, E], F32, tag="embg")
        nc.gpsimd.indirect_dma_start(
            out=emb_t[:], out_offset=None, in_=emb_h[:],
            in_offset=bass.IndirectOffsetOnAxis(ap=idx_t[:, :1], axis=0),
        )

        # =================== phase A: attention ===================
        with tc.tile_pool(name="psA", bufs=2, space="PSUM") as psA:
            # pscore[b] = prev_h[b] . w_h + attn_b
            ttr_o = spool.tile([BL, H], F32, tag="ttro_p")
            nc.vector.tensor_mul(ttr_o[:], ph_t[:], wh_t[:])
            psc0 = spool.tile([BL, 1], F32, tag="psc0")
            nc.vector.reduce_sum(psc0[:], ttr_o[:], axis=mybir.AxisListType.X)
            pscore = spool.tile([BL, 1], F32, tag="pscore")
            nc.vector.tensor_add(pscore[:], psc0[:], ab_t[:])

            # scores_sT[s, b] = sum_h enc[b,s,h] * w_e[h]
            ssT = spool.tile([S, BL], F32, tag="ssT")
            for b in range(BL):
                ttr_e = spool.tile([S, H], F32, tag="ttro_e", bufs=2)
                nc.vector.tensor_mul(
                    ttr_e[:], enc_t[:, b * H:(b + 1) * H], we_t[:])
                nc.vector.reduce_sum(ssT[:, b:b + 1], ttr_e[:],
                                     axis=mybir.AxisListType.X)

            # transpose scores to [BL, S], add pscore, softmax
            sc_ps = psA.tile([BL, S], F32, tag="psa", bufs=2)
            nc.tensor.transpose(out=sc_ps[:], in_=ssT[:], identity=ident[:])
            sc_t = spool.tile([BL, S], F32, tag="scores")
            nc.vector.tensor_scalar_add(sc_t[:], sc_ps[:], pscore[:, :1])
            nmax = spool.tile([BL, 1], F32, tag="nmax")
            nc.vector.reduce_max(nmax[:], sc_t[:], axis=mybir.AxisListType.X,
                                 negate=True)
            expt = spool.tile([BL, S], F32, tag="expt")
            sumex = spool.tile([BL, 1], F32, tag="sumex")
            nc.scalar.activation(expt[:], sc_t[:],
                                 mybir.ActivationFunctionType.Exp,
                                 bias=nmax[:, :1], accum_out=sumex[:, :1])
            rsum = spool.tile([BL, 1], F32, tag="rsum")
            nc.vector.reciprocal(rsum[:], sumex[:])
            attn_t = spool.tile([BL, S], F32, tag="attn")
            nc.vector.tensor_scalar_mul(attn_t[:], expt[:], rsum[:, :1])
            nc.sync.dma_start(out=attn_o[:], in_=attn_t[:])

            # transpose attn back to [S, BL]
            at_ps = psA.tile([S, BL], F32, tag="psa", bufs=2)
            nc.tensor.transpose(out=at_ps[:], in_=attn_t[:],
                                identity=ident[:BL, :BL])
            attnT = spool.tile([S, BL], F32, tag="attnT")
            nc.scalar.copy(attnT[:], at_ps[:])

            # embedding part of xT contribution: transpose [BL,E] -> [E,BL]
            for c in range(E // 128):
                tr_ps = psA.tile([128, BL], F32, tag="psa", bufs=2)
                nc.tensor.transpose(out=tr_ps[:],
                                    in_=emb_t[:, c * 128:(c + 1) * 128],
                                    identity=ident[:BL, :BL])
                tr_sb = spool.tile([128, BL], F32, tag="trsb", bufs=2)
                nc.scalar.copy(tr_sb[:], tr_ps[:])
                nc.sync.dma_start(out=xcon[c * 128:(c + 1) * 128, :],
                                  in_=tr_sb[:])

            # context part: ctxT[h, b] = sum_s attn[b,s] enc[b,s,h]
            for hc in range(KC_H):
                ctx_ps = psA.tile([128, BL], F32, tag="psctx", bufs=2)
                for b in range(BL):
                    nc.tensor.matmul(
                        ctx_ps[:, b:b + 1],
                        enc_t[:, b * H + hc * 128: b * H + (hc + 1) * 128],
                        attnT[:, b:b + 1],
                        start=True, stop=True)
                ctx_sb = spool.tile([128, BL], F32, tag="ctxsb", bufs=2)
                nc.scalar.copy(ctx_sb[:], ctx_ps[:])
                nc.sync.dma_start(out=xcon[E + hc * 128:E + (hc + 1) * 128, :],
                                  in_=ctx_sb[:])

        nc.gpsimd.collective_compute(
            "AllGather", mybir.AluOpType.bypass, replica_groups=groups,
            ins=[xcon[:]], outs=[XG[:]])

        # =================== phase B: LSTM slice ===================
        with tc.tile_pool(name="psB", bufs=4, space="PSUM") as psB:
            xt_t = spool.tile([128, KC_X * B], F32R, tag="xt")
            for c in range(KC_X):
                nc.sync.dma_start(
                    out=xt_t[:, c * B:(c + 1) * B].rearrange(
                        "p (k b) -> p k b", b=BL),
                    in_=XG[:].rearrange("k c b -> c k b")[c * 128:(c + 1) * 128].bitcast(F32R))

            gact = []
            funcs = [mybir.ActivationFunctionType.Sigmoid,      # i
                     mybir.ActivationFunctionType.Sigmoid,      # f
                     mybir.ActivationFunctionType.Tanh,         # g
                     mybir.ActivationFunctionType.Sigmoid]      # o
            for g in range(4):
                ps_g = psB.tile([HL, B], F32, tag="gates", bufs=4)
                for c in range(KC_X):
                    nc.tensor.matmul(
                        ps_g[:],
                        _r(wih_t[:, c * 4 * HL + g * HL: c * 4 * HL + (g + 1) * HL]),
                        _r(xt_t[:, c * B:(c + 1) * B]),
                        start=(c == 0), stop=False)
                for c in range(KC_H):
                    nc.tensor.matmul(
                        ps_g[:],
                        _r(whh_t[:, c * 4 * HL + g * HL: c * 4 * HL + (g + 1) * HL]),
                        _r(phT_t[:, c * B:(c + 1) * B]),
                        start=False, stop=(c == KC_H - 1))
                a_sb = spool.tile([HL, B], F32, tag=f"gate{g}")
                nc.scalar.activation(a_sb[:], ps_g[:], funcs[g],
                                     bias=bias_t[:, g:g + 1])
                gact.append(a_sb)

            t1 = spool.tile([HL, B], F32, tag="t1")
            nc.vector.tensor_mul(t1[:], gact[1][:], pcT_t[:])
            t2 = spool.tile([HL, B], F32, tag="t2")
            nc.vector.tensor_mul(t2[:], gact[0][:], gact[2][:])
            cellT = spool.tile([HL, B], F32, tag="cellT")
            nc.vector.tensor_add(cellT[:], t1[:], t2[:])
            tanhc = spool.tile([HL, B], F32, tag="tanhc")
            nc.scalar.activation(tanhc[:], cellT[:],
                                 mybir.ActivationFunctionType.Tanh)
            hT = spool.tile([HL, B], F32, tag="hT")
            nc.vector.tensor_mul(hT[:], gact[3][:], tanhc[:])

            nc.sync.dma_start(out=cell_o[:], in_=cellT[:])
            nc.sync.dma_start(out=hid_o[:], in_=hT[:])
            nc.sync.dma_start(out=hcon[:], in_=hT[:])

        nc.gpsimd.collective_compute(
            "AllGather", mybir.AluOpType.bypass, replica_groups=groups,
            ins=[hcon[:]], outs=[HG[:]])

        # =================== phase C: vocab projection ===================
        with tc.tile_pool(name="psC", bufs=1, space="PSUM") as psC, \
             tc.tile_pool(name="wtp", bufs=4) as wtp:
            hg_t = spool.tile([128, KC_H * B], F32R, tag="hg")
            for c in range(KC_H):
                nc.sync.dma_start(out=hg_t[:, c * B:(c + 1) * B], in_=HG[c].bitcast(F32R))

            ps_nt = [psC.tile([B, NTW], F32, tag=f"proj{nt}", bufs=1,
                              name=f"ps_proj{nt}")
                     for nt in range(NT)]
            # bias via rank-1 matmul (doesn't depend on hidden)
            for nt in range(NT):
                nc.tensor.matmul(ps_nt[nt][:], _r(ones_row[:]),
                                 _r(wb_t[:, nt * NTW:(nt + 1) * NTW]),
                                 start=True, stop=False)
            for kc in range(KC_H):
                wt_t = wtp.tile([128, VL], F32R, tag="wt", bufs=4)
                nc.sync.dma_start(out=wt_t[:],
                                  in_=owt_h[kc * 128:(kc + 1) * 128, :].bitcast(F32R))
                for nt in range(NT):
                    nc.tensor.matmul(
                        ps_nt[nt][:], _r(hg_t[:, kc * B:(kc + 1) * B]),
                        _r(wt_t[:, nt * NTW:(nt + 1) * NTW]),
                        start=False, stop=(kc == KC_H - 1))

            # local log-softmax stats
            maxs = spool.tile([B, NT], F32, tag="maxs")
            for nt in range(NT):
                nc.vector.reduce_max(maxs[:, nt:nt + 1], ps_nt[nt][:],
                                     axis=mybir.AxisListType.X)
            lmax = spool.tile([B, 1], F32, tag="lmax")
            nc.vector.reduce_max(lmax[:], maxs[:], axis=mybir.AxisListType.X)
            nlmax = spool.tile([B, 1], F32, tag="nlmax")
            nc.scalar.mul(nlmax[:], lmax[:], -1.0)
            sums = spool.tile([B, NT], F32, tag="sums")
            for nt in range(NT):
                esc = spool.tile([B, NTW], F32, tag="esc", bufs=2)
                nc.scalar.activation(esc[:], ps_nt[nt][:],
                                     mybir.ActivationFunctionType.Exp,
                                     bias=nlmax[:, :1],
                                     accum_out=sums[:, nt:nt + 1])
            lsum = spool.tile([B, 1], F32, tag="lsum")
            nc.vector.reduce_sum(lsum[:], sums[:], axis=mybir.AxisListType.X)
            stat = spool.tile([B, 2], F32, tag="stat")
            nc.vector.tensor_copy(stat[:, 0:1], lmax[:])
            nc.vector.tensor_copy(stat[:, 1:2], lsum[:])
            nc.sync.dma_start(out=scon[:], in_=stat[:])

            nc.gpsimd.collective_compute(
                "AllGather", mybir.AluOpType.bypass, replica_groups=groups,
                ins=[scon[:]], outs=[SG[:]])

            # combine stats: gmax = max_k m_k; Z = sum_k s_k * exp(m_k-gmax)
            m_all = spool.tile([B, NCORE], F32, tag="mall")
            s_all = spool.tile([B, NCORE], F32, tag="sall")
            nc.sync.dma_start(out=m_all[:],
                              in_=SG[:].rearrange("k b s -> b s k")[:, 0])
            nc.sync.dma_start(out=s_all[:],
                              in_=SG[:].rearrange("k b s -> b s k")[:, 1])
            gmax = spool.tile([B, 1], F32, tag="gmax")
            nc.vector.reduce_max(gmax[:], m_all[:], axis=mybir.AxisListType.X)
            ngmax = spool.tile([B, 1], F32, tag="ngmax")
            nc.scalar.mul(ngmax[:], gmax[:], -1.0)
            delt = spool.tile([B, NCORE], F32, tag="delt")
            nc.vector.tensor_scalar_add(delt[:], m_all[:], ngmax[:, :1])
            expd = spool.tile([B, NCORE], F32, tag="expd")
            nc.scalar.activation(expd[:], delt[:],
                                 mybir.ActivationFunctionType.Exp)
            terms = spool.tile([B, NCORE], F32, tag="terms")
            nc.vector.tensor_mul(terms[:], expd[:], s_all[:])
            zsum = spool.tile([B, 1], F32, tag="zsum")
            nc.vector.reduce_sum(zsum[:], terms[:], axis=mybir.AxisListType.X)
            lnz = spool.tile([B, 1], F32, tag="lnz")
            nc.scalar.activation(lnz[:], zsum[:],
                                 mybir.ActivationFunctionType.Ln)
            shift = spool.tile([B, 1], F32, tag="shift")
            nc.vector.tensor_add(shift[:], gmax[:], lnz[:])
            nshift = spool.tile([B, 1], F32, tag="nshift")
            nc.scalar.mul(nshift[:], shift[:], -1.0)

            for nt in range(NT):
                o_sb = spool.tile([B, NTW], F32, tag="osb", bufs=3)
                nc.vector.tensor_scalar_add(o_sb[:], ps_nt[nt][:],
                                            nshift[:, :1])
                nc.sync.dma_start(out=log_o[:, nt * NTW:(nt + 1) * NTW],
                                  in_=o_sb[:])

    nc.finalize()
    return nc


def shard_inputs(input_batch, prev_h, prev_c, encoder_outputs, curr_idxs,
                 emb, attn_w, attn_b, w_ih, b_ih, w_hh, b_hh, out_w, out_b):
    f = lambda x: np.ascontiguousarray(np.asarray(x, dtype=np.float32))
    input_batch = np.ascontiguousarray(np.asarray(input_batch, dtype=np.int32))
    emb, attn_w, attn_b = f(emb), f(attn_w), f(attn_b)
    prev_h, prev_c, encoder_outputs = f(prev_h), f(prev_c), f(encoder_outputs)
    w_ih, b_ih, w_hh, b_hh, out_w, out_b = (f(w_ih), f(b_ih), f(w_hh),
                                            f(b_hh), f(out_w), f(out_b))

    phT = np.ascontiguousarray(prev_h.T)                      # [H, B]
    pcT = np.ascontiguousarray(prev_c.T)                      # [H, B]
    we_b = np.ascontiguousarray(np.broadcast_to(attn_w[0, H:], (S, H)))
    wh_b = np.ascontiguousarray(np.broadcast_to(attn_w[0, :H], (BL, H)))
    ab_b = np.full((BL, 1), attn_b[0], np.float32)
    wihT = np.ascontiguousarray(w_ih.T)                       # [EH, 4H]
    whhT = np.ascontiguousarray(w_hh.T)                       # [H, 4H]
    bsum = b_ih + b_hh                                        # [4H]
    owT = np.ascontiguousarray(out_w.T)                       # [H, V]

    in_maps = []
    for k in range(NCORE):
        bs = slice(k * BL, (k + 1) * BL)
        hs = slice(k * HL, (k + 1) * HL)
        wih_loc = np.ascontiguousarray(np.concatenate(
            [wihT[:, g * H + k * HL: g * H + (k + 1) * HL] for g in range(4)],
            axis=1))
        whh_loc = np.ascontiguousarray(np.concatenate(
            [whhT[:, g * H + k * HL: g * H + (k + 1) * HL] for g in range(4)],
            axis=1))
        bias_loc = np.ascontiguousarray(np.stack(
            [bsum[g * H + k * HL: g * H + (k + 1) * HL] for g in range(4)],
            axis=1))
        owtb_loc = np.ascontiguousarray(np.concatenate(
            [owT[:, k * VL:(k + 1) * VL],
             out_b[None, k * VL:(k + 1) * VL]], axis=0))
        in_maps.append({
            "tok": input_batch[bs],
            "emb": emb,
            "enc": encoder_outputs[bs],
            "ph_row": prev_h[bs],
            "phT": phT,
            "pcT_loc": np.ascontiguousarray(pcT[hs]),
            "we_b": we_b,
            "wh_b": wh_b,
            "ab_b": ab_b,
            "wihT_loc": wih_loc,
            "whhT_loc": whh_loc,
            "bias_loc": bias_loc,
            "owTb_loc": owtb_loc,
            "ones": np.ones((1, B), np.float32),
        })
    return in_maps


def assemble_outputs(results):
    output = np.concatenate([r["logits_out"] for r in results], axis=1)
    hidden = np.concatenate([r["hidden_out"] for r in results], axis=0).T
    cell = np.concatenate([r["cell_out"] for r in results], axis=0).T
    attn = np.concatenate([r["attn_out"] for r in results], axis=0)
    return (np.ascontiguousarray(output), np.ascontiguousarray(hidden),
            np.ascontiguousarray(cell), np.ascontiguousarray(attn))


def get_program():
    if "nc" not in _prog_cache:
        _prog_cache["nc"] = build_program()
    return _prog_cache["nc"]


def run(trace=False, **inputs):
    nc = get_program()
    in_maps = shard_inputs(**inputs)
    res = run_bass_kernel_spmd(nc, in_maps, list(range(NCORE)), trace=trace)
    return assemble_outputs(res.results), res


def kernel(**inputs):
    outs, _ = run(trace=False, **inputs)
    return outs
